# revision 1
# baseline (speedup 1.0000x reference)
"""Cross-attention (RoPE, H=8, D=64) Trainium2 kernel, 8-core SPMD.

Sharding: core i handles batch b = i//4 and head-pair p = i%4
(heads 2p, 2p+1  ==  channel slice [128p : 128p+128) of the 512-dim space).
Each core computes, for its batch and its 2 heads:
    K.T / Q.T projections (+bias +RoPE)  -> [128, 4096] bf16 (2 heads stacked)
    V projection (+bias)                 -> per-head [128m-chunk, 64] bf16 (+ones col)
    flash-style attention with scores kept transposed (S.T = [m, n]):
        S.T = K.T_h^T-free-chunked matmuls, exp on ScalarE (scale fused),
        AV + softmax denominator in one matmul via the ones column of V',
        normalization via reciprocal + DRAM-roundtrip partition broadcast.
    partial output projection: att.T[slice] @ Wo.T[slice] -> [4096, 512] f32
Host sums the 4 partials per batch and adds bo.

All matmuls run in bf16 (full PE rate); accumulation is fp32 in PSUM.
"""

import sys

if "/opt/trn_rl_repo" not in sys.path:
    sys.path.insert(0, "/opt/trn_rl_repo")

from contextlib import ExitStack

import numpy as np
import ml_dtypes

import concourse.tile as tile
from concourse import bacc, mybir
from concourse.bass_utils import run_bass_kernel_spmd

F32 = mybir.dt.float32
BF16 = mybir.dt.bfloat16
EXP = mybir.ActivationFunctionType.Exp

B, N, C = 2, 4096, 512
H, D = 8, 64
M = 4096
SCALE = float(D) ** -0.5
ROPE_BASE = 10000.0
NCORES = 8
PJ = 128          # channels per core (2 heads)
MB = M // 512     # 8  kv blocks of 512
NB = N // 512     # 8  query blocks of 512
MC = M // 128     # 32 key chunks of 128


def _build(tc, aps):
    nc = tc.nc
    (xT, ctxT, wqT, wkT, wvT, woT, bqT, bkT, bv, cosT, sinT, r2T, scr, out) = aps
    es = ExitStack()
    with es:
        const = es.enter_context(tc.tile_pool(name="const", bufs=1))
        resid = es.enter_context(tc.tile_pool(name="resid", bufs=1))

        # ---- constants ----
        wq_sb = const.tile([128, 4, PJ], BF16)
        nc.sync.dma_start(wq_sb[:], wqT.rearrange("(o p) j -> p o j", p=128))
        wk_sb = const.tile([128, 4, PJ], BF16)
        nc.sync.dma_start(wk_sb[:], wkT.rearrange("(o p) j -> p o j", p=128))
        wv_sb = const.tile([128, 4, PJ], BF16)
        nc.sync.dma_start(wv_sb[:], wvT.rearrange("(o p) j -> p o j", p=128))
        wo_sb = const.tile([128, C], BF16)
        nc.sync.dma_start(wo_sb[:], woT)
        bq_sb = const.tile([128, 1], F32)
        nc.sync.dma_start(bq_sb[:], bqT)
        bk_sb = const.tile([128, 1], F32)
        nc.sync.dma_start(bk_sb[:], bkT)
        bv_sb = const.tile([128, PJ], F32)
        nc.sync.dma_start(bv_sb[:], bv.to_broadcast((128, PJ)))
        r2_sb = const.tile([128, 128], BF16)
        nc.sync.dma_start(r2_sb[:], r2T)
        cos_sb = const.tile([128, N], F32)
        nc.sync.dma_start(cos_sb[:], cosT)
        sin_sb = const.tile([128, N], F32)
        nc.sync.dma_start(sin_sb[:], sinT)

        # ---- residents ----
        KT = resid.tile([128, M], BF16)     # roped K.T, 2 heads stacked on partitions
        QT = resid.tile([128, N], BF16)
        Vp0 = resid.tile([128, MC, 65], BF16)  # [m-in-chunk, m-chunk, V|ones]
        Vp1 = resid.tile([128, MC, 65], BF16)
        ATT = resid.tile([128, N], BF16)    # normalized attention output, transposed
        nc.vector.memset(Vp0[:, :, 64:65], 1.0)
        nc.vector.memset(Vp1[:, :, 64:65], 1.0)

        # ---- K/V and Q projections (+bias, +RoPE for K/Q) ----
        with (
            tc.tile_pool(name="pw", bufs=3) as work,
            tc.tile_pool(name="pp", bufs=2, space="PSUM") as pp,
        ):
            def proj_rope(src_ap, w_sb, b_sb, dst, blk, with_v):
                sl = slice(512 * blk, 512 * blk + 512)
                act = work.tile([128, 4, 512], BF16, tag="act")
                nc.sync.dma_start(act[:], src_ap.rearrange("(o p) m -> p o m", p=128)[:, :, sl])
                ps = pp.tile([128, 512], F32, tag="ps")
                for c in range(4):
                    nc.tensor.matmul(ps[:], w_sb[:, c, :], act[:, c, :],
                                     start=(c == 0), stop=(c == 3))
                kb = work.tile([128, 512], BF16, tag="kb")
                nc.vector.tensor_add(out=kb[:], in0=ps[:], in1=b_sb.to_broadcast((128, 512)))
                pr = pp.tile([128, 512], F32, tag="pr")
                nc.tensor.matmul(pr[:], r2_sb[:], kb[:], start=True, stop=True)
                t1 = work.tile([128, 512], F32, tag="t1")
                nc.vector.tensor_mul(out=t1[:], in0=kb[:], in1=cos_sb[:, sl])
                t2 = work.tile([128, 512], F32, tag="t2")
                nc.vector.tensor_mul(out=t2[:], in0=pr[:], in1=sin_sb[:, sl])
                nc.vector.tensor_add(out=dst[:, sl], in0=t1[:], in1=t2[:])
                if with_v:
                    for mm in range(4):
                        pv = pp.tile([128, 128], F32, tag="pv")
                        for c in range(4):
                            nc.tensor.matmul(pv[:], act[:, c, 128 * mm:128 * mm + 128],
                                             wv_sb[:, c, :], start=(c == 0), stop=(c == 3))
                        mci = 4 * blk + mm
                        nc.vector.tensor_add(out=Vp0[:, mci, 0:64], in0=pv[:, 0:64],
                                             in1=bv_sb[:, 0:64])
                        nc.vector.tensor_add(out=Vp1[:, mci, 0:64], in0=pv[:, 64:128],
                                             in1=bv_sb[:, 64:128])

            for mb in range(MB):
                proj_rope(ctxT, wk_sb, bk_sb, KT, mb, with_v=True)
            for nb in range(NB):
                proj_rope(xT, wq_sb, bq_sb, QT, nb, with_v=False)

        # ---- attention + output projection ----
        with (
            tc.tile_pool(name="ew", bufs=6) as ew,
            tc.tile_pool(name="nw", bufs=2) as nw,
            tc.tile_pool(name="sp", bufs=4, space="PSUM") as sp,
            tc.tile_pool(name="vp", bufs=2, space="PSUM") as vp,
            tc.tile_pool(name="op", bufs=2, space="PSUM") as op,
        ):
            for nb in range(NB):
                nsl = slice(512 * nb, 512 * nb + 512)
                pv0 = vp.tile([128, 512], F32, tag="pv")
                pv1 = vp.tile([128, 512], F32, tag="pv")
                for mc in range(MC):
                    mcs = slice(128 * mc, 128 * mc + 128)
                    ps0 = sp.tile([128, 512], F32, tag="ps")
                    ps1 = sp.tile([128, 512], F32, tag="ps")
                    nc.tensor.matmul(ps0[:], KT[0:64, mcs], QT[0:64, nsl],
                                     start=True, stop=True, tile_position=(0, 0))
                    nc.tensor.matmul(ps1[:], KT[64:128, mcs], QT[64:128, nsl],
                                     start=True, stop=True, tile_position=(64, 0))
                    e0 = ew.tile([128, 512], BF16, tag="e0")
                    nc.scalar.activation(e0[:], ps0[:], EXP, scale=SCALE)
                    e1 = ew.tile([128, 512], BF16, tag="e1")
                    nc.scalar.activation(e1[:], ps1[:], EXP, scale=SCALE)
                    nc.tensor.matmul(pv0[0:65, :], Vp0[:, mc, :], e0[:],
                                     start=(mc == 0), stop=(mc == MC - 1))
                    nc.tensor.matmul(pv1[0:65, :], Vp1[:, mc, :], e1[:],
                                     start=(mc == 0), stop=(mc == MC - 1))
                # normalize:  att = num * (1/denom), denom broadcast via DRAM roundtrip
                rec0 = nw.tile([128, 512], F32, tag="rec0")
                nc.vector.reciprocal(rec0[64:65, :], pv0[64:65, :])
                rec1 = nw.tile([128, 512], F32, tag="rec1")
                nc.vector.reciprocal(rec1[64:65, :], pv1[64:65, :])
                nc.sync.dma_start(scr[2 * nb:2 * nb + 1, :], rec0[64:65, :])
                nc.sync.dma_start(scr[2 * nb + 1:2 * nb + 2, :], rec1[64:65, :])
                bc = nw.tile([128, 512], F32, tag="bc")
                nc.sync.dma_start(bc[0:64, :], scr[2 * nb:2 * nb + 1, :].to_broadcast((64, 512)))
                nc.sync.dma_start(bc[64:128, :], scr[2 * nb + 1:2 * nb + 2, :].to_broadcast((64, 512)))
                nc.vector.tensor_mul(out=ATT[0:64, nsl], in0=pv0[0:64, :], in1=bc[0:64, :])
                nc.vector.tensor_mul(out=ATT[64:128, nsl], in0=pv1[0:64, :], in1=bc[64:128, :])
                # output projection for this query block
                for nn in range(4):
                    rsl = slice(512 * nb + 128 * nn, 512 * nb + 128 * nn + 128)
                    po = op.tile([128, 512], F32, tag="po")
                    nc.tensor.matmul(po[:], ATT[:, rsl], wo_sb[:], start=True, stop=True)
                    ob = nw.tile([128, 512], F32, tag="ob")
                    nc.vector.tensor_copy(out=ob[:], in_=po[:])
                    nc.sync.dma_start(out[rsl, :], ob[:])


def build_program():
    nc = bacc.Bacc("TRN2", target_bir_lowering=False, debug=False)

    def din(name, shape, dt):
        return nc.dram_tensor(name, shape, dt, kind="ExternalInput").ap()

    aps = (
        din("xT", [C, N], BF16),
        din("ctxT", [C, M], BF16),
        din("wqT", [C, PJ], BF16),
        din("wkT", [C, PJ], BF16),
        din("wvT", [C, PJ], BF16),
        din("woT", [PJ, C], BF16),
        din("bqT", [PJ, 1], F32),
        din("bkT", [PJ, 1], F32),
        din("bv", [1, PJ], F32),
        din("cosT", [PJ, N], F32),
        din("sinT", [PJ, N], F32),
        din("r2T", [PJ, PJ], BF16),
        nc.dram_tensor("scr", [2 * NB, 512], F32).ap(),
        nc.dram_tensor("out", [N, C], F32, kind="ExternalOutput").ap(),
    )
    with tile.TileContext(nc) as tc:
        _build(tc, aps)
    nc.compile()
    return nc


_PROG = None


def _program():
    global _PROG
    if _PROG is None:
        _PROG = build_program()
    return _PROG


def rope_tables():
    idx = np.arange(0, D, 2, dtype=np.float32)
    inv_freq = 1.0 / (ROPE_BASE ** (idx / D))
    t = np.arange(N, dtype=np.float32)
    freqs = t[:, None] * inv_freq[None, :]          # (N, 32)
    emb = np.concatenate([freqs, freqs], axis=1)    # (N, 64)
    cos64 = np.cos(emb).T.astype(np.float32)        # (64, N)
    sin64 = np.sin(emb).T.astype(np.float32)
    cosT = np.ascontiguousarray(np.vstack([cos64, cos64]))
    sinT = np.ascontiguousarray(np.vstack([sin64, sin64]))
    return cosT, sinT


def r2t_matrix():
    R = np.zeros((D, D), np.float32)
    for i in range(D // 2):
        R[2 * i, 2 * i + 1] = -1.0
        R[2 * i + 1, 2 * i] = 1.0
    R2 = np.zeros((PJ, PJ), np.float32)
    R2[0:D, 0:D] = R
    R2[D:PJ, D:PJ] = R
    return np.ascontiguousarray(R2.T).astype(ml_dtypes.bfloat16)


def make_in_maps(x, context, Wq, bq, Wk, bk, Wv, bv, Wo):
    def bf(a):
        return np.ascontiguousarray(a).astype(ml_dtypes.bfloat16)

    def f32c(a):
        return np.ascontiguousarray(a, dtype=np.float32)

    cosT, sinT = rope_tables()
    r2T = r2t_matrix()
    xTb = [bf(x[b].T) for b in range(B)]
    ctxTb = [bf(context[b].T) for b in range(B)]
    in_maps = []
    for core in range(NCORES):
        b, p = core // 4, core % 4
        sl = slice(PJ * p, PJ * p + PJ)
        in_maps.append({
            "xT": xTb[b],
            "ctxT": ctxTb[b],
            "wqT": bf(Wq[sl, :].T),
            "wkT": bf(Wk[sl, :].T),
            "wvT": bf(Wv[sl, :].T),
            "woT": bf(Wo[:, sl].T),
            "bqT": f32c(bq[sl].reshape(PJ, 1)),
            "bkT": f32c(bk[sl].reshape(PJ, 1)),
            "bv": f32c(bv[sl].reshape(1, PJ)),
            "cosT": cosT,
            "sinT": sinT,
            "r2T": r2T,
        })
    return in_maps


def gather(partials, bo):
    final = np.empty((B, N, C), np.float32)
    for b in range(B):
        acc = partials[4 * b].astype(np.float32).copy()
        for p in range(1, 4):
            acc += partials[4 * b + p]
        final[b] = acc + np.asarray(bo, np.float32)[None, :]
    return final


def kernel(x, context, Wq, bq, Wk, bk, Wv, bv, Wo, bo, **kw):
    x = np.asarray(x, np.float32)
    context = np.asarray(context, np.float32)
    nc = _program()
    in_maps = make_in_maps(x, context, np.asarray(Wq, np.float32), np.asarray(bq, np.float32),
                           np.asarray(Wk, np.float32), np.asarray(bk, np.float32),
                           np.asarray(Wv, np.float32), np.asarray(bv, np.float32),
                           np.asarray(Wo, np.float32))
    res = run_bass_kernel_spmd(nc, in_maps, list(range(NCORES)))
    partials = [res.results[i]["out"] for i in range(NCORES)]
    return gather(partials, np.asarray(bo, np.float32))


# revision 5
# speedup vs baseline: 1.2056x; 1.2056x over previous
"""Cross-attention (RoPE, H=8, D=64) Trainium2 kernel, 8-core SPMD.

Sharding: core i handles batch b = i//4 and head-pair p = i%4
(heads 2p, 2p+1  ==  channel slice [128p : 128p+128) of the 512-dim space).
Each core computes, for its batch and its 2 heads:
    K.T / Q.T projections (+bias +RoPE)  -> [128, 4096] bf16 (2 heads stacked)
    V projection (+bias)                 -> per-head [128m-chunk, 64] bf16 (+ones col)
    flash-style attention with scores kept transposed (S.T = [m, n]):
        S.T = K.T_h^T-free-chunked matmuls, exp on ScalarE (scale fused),
        AV + softmax denominator in one matmul via the ones column of V',
        normalization via reciprocal + DRAM-roundtrip partition broadcast.
    partial output projection: att.T[slice] @ Wo.T[slice] -> [4096, 512] f32
Host sums the 4 partials per batch and adds bo.

All matmuls run in bf16 (full PE rate); accumulation is fp32 in PSUM.
"""

import sys

if "/opt/trn_rl_repo" not in sys.path:
    sys.path.insert(0, "/opt/trn_rl_repo")

from contextlib import ExitStack

import numpy as np
import ml_dtypes

import concourse.tile as tile
from concourse import bacc, mybir
from concourse.bass_utils import run_bass_kernel_spmd

F32 = mybir.dt.float32
BF16 = mybir.dt.bfloat16
EXP = mybir.ActivationFunctionType.Exp

B, N, C = 2, 4096, 512
H, D = 8, 64
M = 4096
SCALE = float(D) ** -0.5
ROPE_BASE = 10000.0
NCORES = 8
PJ = 128          # channels per core (2 heads)
MB = M // 512     # 8  kv blocks of 512
NB = N // 512     # 8  query blocks of 512
MC = M // 128     # 32 key chunks of 128


def _build(tc, aps):
    nc = tc.nc
    (xT, ctxT, wqT, wkT, wvT, woT, bqT, bkT, bv, cosT, sinT, r2T, scr, out) = aps
    es = ExitStack()
    with es:
        const = es.enter_context(tc.tile_pool(name="const", bufs=1))
        resid = es.enter_context(tc.tile_pool(name="resid", bufs=1))

        # ---- constants ----
        wq_sb = const.tile([128, 4, PJ], BF16)
        nc.sync.dma_start(wq_sb[:], wqT.rearrange("(o p) j -> p o j", p=128))
        wk_sb = const.tile([128, 4, PJ], BF16)
        nc.sync.dma_start(wk_sb[:], wkT.rearrange("(o p) j -> p o j", p=128))
        wv_sb = const.tile([128, 4, PJ], BF16)
        nc.sync.dma_start(wv_sb[:], wvT.rearrange("(o p) j -> p o j", p=128))
        wo_sb = const.tile([128, C], BF16)
        nc.sync.dma_start(wo_sb[:], woT)
        bq_sb = const.tile([128, 1], F32)
        nc.sync.dma_start(bq_sb[:], bqT)
        bk_sb = const.tile([128, 1], F32)
        nc.sync.dma_start(bk_sb[:], bkT)
        bv_sb = const.tile([128, PJ], F32)
        nc.sync.dma_start(bv_sb[:], bv.to_broadcast((128, PJ)))
        r2_sb = const.tile([128, 128], BF16)
        nc.sync.dma_start(r2_sb[:], r2T)
        cos_sb = const.tile([128, N], F32)
        nc.sync.dma_start(cos_sb[:], cosT)
        sin_sb = const.tile([128, N], F32)
        nc.sync.dma_start(sin_sb[:], sinT)

        # ---- residents ----
        KT = resid.tile([128, M], BF16)     # roped K.T, 2 heads stacked on partitions
        Vp0 = resid.tile([128, MC, 65], BF16)  # [m-in-chunk, m-chunk, V|ones]
        Vp1 = resid.tile([128, MC, 65], BF16)
        nc.vector.memset(Vp0[:, :, 64:65], 1.0)
        nc.vector.memset(Vp1[:, :, 64:65], 1.0)

        # ---- shared pools (PSUM budget: ps 2x2 + pv 2 + po 2 = 8 banks) ----
        with (
            tc.tile_pool(name="pw", bufs=3) as work,
            tc.tile_pool(name="ew", bufs=8) as ew,
            tc.tile_pool(name="nw", bufs=2) as nw,
            tc.tile_pool(name="sp", bufs=2, space="PSUM") as sp,
            tc.tile_pool(name="vp", bufs=2, space="PSUM") as vp,
            tc.tile_pool(name="op", bufs=2, space="PSUM") as op,
            tc.tile_pool(name="qp", bufs=3) as qpool,
            tc.tile_pool(name="ap", bufs=2) as apool,
        ):
            def proj_rope(src_ap, w_sb, b_sb, dst, dsl, blk, with_v):
                sl = slice(512 * blk, 512 * blk + 512)
                act = work.tile([128, 4, 512], BF16, tag="act")
                nc.sync.dma_start(act[:], src_ap.rearrange("(o p) m -> p o m", p=128)[:, :, sl])
                ps = op.tile([128, 512], F32, tag="po")
                for c in range(4):
                    nc.tensor.matmul(ps[:], w_sb[:, c, :], act[:, c, :],
                                     start=(c == 0), stop=(c == 3))
                kb = work.tile([128, 512], BF16, tag="kb")
                nc.vector.tensor_add(out=kb[:], in0=ps[:], in1=b_sb.to_broadcast((128, 512)))
                pr = op.tile([128, 512], F32, tag="po")
                nc.tensor.matmul(pr[:], r2_sb[:], kb[:], start=True, stop=True)
                t1 = work.tile([128, 512], F32, tag="t1")
                nc.vector.tensor_mul(out=t1[:], in0=kb[:], in1=cos_sb[:, sl])
                t2 = work.tile([128, 512], F32, tag="t2")
                nc.vector.tensor_mul(out=t2[:], in0=pr[:], in1=sin_sb[:, sl])
                nc.vector.tensor_add(out=dst[:, dsl], in0=t1[:], in1=t2[:])
                if with_v:
                    for mm in range(4):
                        pv = op.tile([128, 128], F32, tag="po")
                        for c in range(4):
                            nc.tensor.matmul(pv[:], act[:, c, 128 * mm:128 * mm + 128],
                                             wv_sb[:, c, :], start=(c == 0), stop=(c == 3))
                        mci = 4 * blk + mm
                        nc.vector.tensor_add(out=Vp0[:, mci, 0:64], in0=pv[:, 0:64],
                                             in1=bv_sb[:, 0:64])
                        nc.vector.tensor_add(out=Vp1[:, mci, 0:64], in0=pv[:, 64:128],
                                             in1=bv_sb[:, 64:128])

            for mb in range(MB):
                proj_rope(ctxT, wk_sb, bk_sb, KT, slice(512 * mb, 512 * mb + 512),
                          mb, with_v=True)
            qts = {}
            qts[0] = qpool.tile([128, 512], BF16, tag="qt", name="qt0")
            proj_rope(xT, wq_sb, bq_sb, qts[0], slice(0, 512), 0, with_v=False)

            # ---- attention + output projection, software-pipelined ----
            for nb in range(NB):
                qt = qts.pop(nb)
                att = apool.tile([128, 512], BF16, tag="att")
                pv0 = vp.tile([128, 512], F32, tag="pv")
                pv1 = vp.tile([128, 512], F32, tag="pv")
                prev = None
                for mc in range(MC + 1):
                    ps01 = None
                    if mc < MC:
                        mcs = slice(128 * mc, 128 * mc + 128)
                        ps01 = sp.tile([128, 1024], F32, tag="ps")
                        nc.tensor.matmul(ps01[:, 0:512], KT[0:64, mcs], qt[0:64, :],
                                         start=True, stop=True, tile_position=(0, 0))
                        nc.tensor.matmul(ps01[:, 512:1024], KT[64:128, mcs], qt[64:128, :],
                                         start=True, stop=True, tile_position=(64, 0))
                    if prev is not None:
                        pmc, pps = prev
                        e01 = ew.tile([128, 1024], BF16, tag="e")
                        nc.scalar.activation(e01[:], pps[:], EXP, scale=SCALE)
                        nc.tensor.matmul(pv0[0:65, :], Vp0[:, pmc, :], e01[:, 0:512],
                                         start=(pmc == 0), stop=(pmc == MC - 1))
                        nc.tensor.matmul(pv1[0:65, :], Vp1[:, pmc, :], e01[:, 512:1024],
                                         start=(pmc == 0), stop=(pmc == MC - 1))
                    # next query block's projection, tucked mid-loop to fill PE idle
                    if mc == 4 and nb + 1 < NB:
                        qts[nb + 1] = qpool.tile([128, 512], BF16, tag="qt", name=f"qt{nb+1}")
                        proj_rope(xT, wq_sb, bq_sb, qts[nb + 1], slice(0, 512),
                                  nb + 1, with_v=False)
                    prev = (mc, ps01) if mc < MC else None
                # normalize:  att = num * (1/denom), denom broadcast via DRAM roundtrip
                rec0 = nw.tile([128, 512], F32, tag="rec0")
                nc.vector.reciprocal(rec0[64:65, :], pv0[64:65, :])
                rec1 = nw.tile([128, 512], F32, tag="rec1")
                nc.vector.reciprocal(rec1[64:65, :], pv1[64:65, :])
                nc.sync.dma_start(scr[2 * nb:2 * nb + 1, :], rec0[64:65, :])
                nc.sync.dma_start(scr[2 * nb + 1:2 * nb + 2, :], rec1[64:65, :])
                bc = nw.tile([128, 512], F32, tag="bc")
                nc.sync.dma_start(bc[0:64, :], scr[2 * nb:2 * nb + 1, :].to_broadcast((64, 512)))
                nc.sync.dma_start(bc[64:128, :], scr[2 * nb + 1:2 * nb + 2, :].to_broadcast((64, 512)))
                nc.vector.tensor_mul(out=att[0:64, :], in0=pv0[0:64, :], in1=bc[0:64, :])
                nc.vector.tensor_mul(out=att[64:128, :], in0=pv1[0:64, :], in1=bc[64:128, :])
                # output projection for this query block
                for nn in range(4):
                    rsl = slice(512 * nb + 128 * nn, 512 * nb + 128 * nn + 128)
                    po = op.tile([128, 512], F32, tag="po")
                    nc.tensor.matmul(po[:], att[:, 128 * nn:128 * nn + 128], wo_sb[:],
                                     start=True, stop=True)
                    ob = nw.tile([128, 512], F32, tag="ob")
                    nc.vector.tensor_copy(out=ob[:], in_=po[:])
                    nc.sync.dma_start(out[rsl, :], ob[:])


def build_program():
    nc = bacc.Bacc("TRN2", target_bir_lowering=False, debug=False)

    def din(name, shape, dt):
        return nc.dram_tensor(name, shape, dt, kind="ExternalInput").ap()

    aps = (
        din("xT", [C, N], BF16),
        din("ctxT", [C, M], BF16),
        din("wqT", [C, PJ], BF16),
        din("wkT", [C, PJ], BF16),
        din("wvT", [C, PJ], BF16),
        din("woT", [PJ, C], BF16),
        din("bqT", [PJ, 1], F32),
        din("bkT", [PJ, 1], F32),
        din("bv", [1, PJ], F32),
        din("cosT", [PJ, N], F32),
        din("sinT", [PJ, N], F32),
        din("r2T", [PJ, PJ], BF16),
        nc.dram_tensor("scr", [2 * NB, 512], F32).ap(),
        nc.dram_tensor("out", [N, C], F32, kind="ExternalOutput").ap(),
    )
    with tile.TileContext(nc) as tc:
        _build(tc, aps)
    nc.compile()
    return nc


_PROG = None


def _program():
    global _PROG
    if _PROG is None:
        _PROG = build_program()
    return _PROG


def rope_tables():
    idx = np.arange(0, D, 2, dtype=np.float32)
    inv_freq = 1.0 / (ROPE_BASE ** (idx / D))
    t = np.arange(N, dtype=np.float32)
    freqs = t[:, None] * inv_freq[None, :]          # (N, 32)
    emb = np.concatenate([freqs, freqs], axis=1)    # (N, 64)
    cos64 = np.cos(emb).T.astype(np.float32)        # (64, N)
    sin64 = np.sin(emb).T.astype(np.float32)
    cosT = np.ascontiguousarray(np.vstack([cos64, cos64]))
    sinT = np.ascontiguousarray(np.vstack([sin64, sin64]))
    return cosT, sinT


def r2t_matrix():
    R = np.zeros((D, D), np.float32)
    for i in range(D // 2):
        R[2 * i, 2 * i + 1] = -1.0
        R[2 * i + 1, 2 * i] = 1.0
    R2 = np.zeros((PJ, PJ), np.float32)
    R2[0:D, 0:D] = R
    R2[D:PJ, D:PJ] = R
    return np.ascontiguousarray(R2.T).astype(ml_dtypes.bfloat16)


def make_in_maps(x, context, Wq, bq, Wk, bk, Wv, bv, Wo):
    def bf(a):
        return np.ascontiguousarray(a).astype(ml_dtypes.bfloat16)

    def f32c(a):
        return np.ascontiguousarray(a, dtype=np.float32)

    cosT, sinT = rope_tables()
    r2T = r2t_matrix()
    xTb = [bf(x[b].T) for b in range(B)]
    ctxTb = [bf(context[b].T) for b in range(B)]
    in_maps = []
    for core in range(NCORES):
        b, p = core // 4, core % 4
        sl = slice(PJ * p, PJ * p + PJ)
        in_maps.append({
            "xT": xTb[b],
            "ctxT": ctxTb[b],
            "wqT": bf(Wq[sl, :].T),
            "wkT": bf(Wk[sl, :].T),
            "wvT": bf(Wv[sl, :].T),
            "woT": bf(Wo[:, sl].T),
            "bqT": f32c(bq[sl].reshape(PJ, 1)),
            "bkT": f32c(bk[sl].reshape(PJ, 1)),
            "bv": f32c(bv[sl].reshape(1, PJ)),
            "cosT": cosT,
            "sinT": sinT,
            "r2T": r2T,
        })
    return in_maps


def gather(partials, bo):
    final = np.empty((B, N, C), np.float32)
    for b in range(B):
        acc = partials[4 * b].astype(np.float32).copy()
        for p in range(1, 4):
            acc += partials[4 * b + p]
        final[b] = acc + np.asarray(bo, np.float32)[None, :]
    return final


def kernel(x, context, Wq, bq, Wk, bk, Wv, bv, Wo, bo, **kw):
    x = np.asarray(x, np.float32)
    context = np.asarray(context, np.float32)
    nc = _program()
    in_maps = make_in_maps(x, context, np.asarray(Wq, np.float32), np.asarray(bq, np.float32),
                           np.asarray(Wk, np.float32), np.asarray(bk, np.float32),
                           np.asarray(Wv, np.float32), np.asarray(bv, np.float32),
                           np.asarray(Wo, np.float32))
    res = run_bass_kernel_spmd(nc, in_maps, list(range(NCORES)))
    partials = [res.results[i]["out"] for i in range(NCORES)]
    return gather(partials, np.asarray(bo, np.float32))


# revision 6
# speedup vs baseline: 1.2292x; 1.0196x over previous
"""Cross-attention (RoPE, H=8, D=64) Trainium2 kernel, 8-core SPMD.

Sharding: core i handles batch b = i//4 and head-pair p = i%4
(heads 2p, 2p+1  ==  channel slice [128p : 128p+128) of the 512-dim space).
Each core computes, for its batch and its 2 heads:
    K.T / Q.T projections (+bias +RoPE)  -> [128, 4096] bf16 (2 heads stacked)
    V projection (+bias)                 -> per-head [128m-chunk, 64] bf16 (+ones col)
    flash-style attention with scores kept transposed (S.T = [m, n]):
        S.T = K.T_h^T-free-chunked matmuls, exp on ScalarE (scale fused),
        AV + softmax denominator in one matmul via the ones column of V',
        normalization via reciprocal + DRAM-roundtrip partition broadcast.
    partial output projection: att.T[slice] @ Wo.T[slice] -> [4096, 512] f32
Host sums the 4 partials per batch and adds bo.

All matmuls run in bf16 (full PE rate); accumulation is fp32 in PSUM.
"""

import sys

if "/opt/trn_rl_repo" not in sys.path:
    sys.path.insert(0, "/opt/trn_rl_repo")

from contextlib import ExitStack

import numpy as np
import ml_dtypes

import concourse.tile as tile
from concourse import bacc, mybir
from concourse.bass_utils import run_bass_kernel_spmd

F32 = mybir.dt.float32
BF16 = mybir.dt.bfloat16
EXP = mybir.ActivationFunctionType.Exp

B, N, C = 2, 4096, 512
H, D = 8, 64
M = 4096
SCALE = float(D) ** -0.5
ROPE_BASE = 10000.0
NCORES = 8
PJ = 128          # channels per core (2 heads)
MB = M // 512     # 8  kv blocks of 512
NB = N // 512     # 8  query blocks of 512
MC = M // 128     # 32 key chunks of 128


def _build(tc, aps):
    nc = tc.nc
    (xT, ctxT, wqT, wkT, wvT, woT, bqT, bkT, bv, cosT, sinT, r2T, scr, out) = aps
    es = ExitStack()
    with es:
        const = es.enter_context(tc.tile_pool(name="const", bufs=1))
        resid = es.enter_context(tc.tile_pool(name="resid", bufs=1))

        # ---- constants ----
        wq_sb = const.tile([128, 4, PJ], BF16)
        nc.sync.dma_start(wq_sb[:], wqT.rearrange("(o p) j -> p o j", p=128))
        wk_sb = const.tile([128, 4, PJ], BF16)
        nc.sync.dma_start(wk_sb[:], wkT.rearrange("(o p) j -> p o j", p=128))
        wv_sb = const.tile([128, 4, PJ], BF16)
        nc.sync.dma_start(wv_sb[:], wvT.rearrange("(o p) j -> p o j", p=128))
        wo_sb = const.tile([128, C], BF16)
        nc.sync.dma_start(wo_sb[:], woT)
        bq_sb = const.tile([128, 1], F32)
        nc.sync.dma_start(bq_sb[:], bqT)
        bk_sb = const.tile([128, 1], F32)
        nc.sync.dma_start(bk_sb[:], bkT)
        bv_sb = const.tile([128, PJ], F32)
        nc.sync.dma_start(bv_sb[:], bv.to_broadcast((128, PJ)))
        r2_sb = const.tile([128, 128], BF16)
        nc.sync.dma_start(r2_sb[:], r2T)
        cos_sb = const.tile([128, N], F32)
        nc.sync.dma_start(cos_sb[:], cosT)
        sin_sb = const.tile([128, N], F32)
        nc.sync.dma_start(sin_sb[:], sinT)

        # ---- residents ----
        KT = resid.tile([128, M], BF16)     # roped K.T, 2 heads stacked on partitions
        Vp0 = resid.tile([128, MC, 65], BF16)  # [m-in-chunk, m-chunk, V|ones]
        Vp1 = resid.tile([128, MC, 65], BF16)
        nc.vector.memset(Vp0[:, :, 64:65], 1.0)
        nc.vector.memset(Vp1[:, :, 64:65], 1.0)

        # ---- shared pools (PSUM budget: ps 2x2 + pv 2 + po 2 = 8 banks) ----
        with (
            tc.tile_pool(name="pw", bufs=3) as work,
            tc.tile_pool(name="ew", bufs=8) as ew,
            tc.tile_pool(name="nw", bufs=2) as nw,
            tc.tile_pool(name="sp", bufs=2, space="PSUM") as sp,
            tc.tile_pool(name="vp", bufs=2, space="PSUM") as vp,
            tc.tile_pool(name="op", bufs=2, space="PSUM") as op,
            tc.tile_pool(name="qp", bufs=3) as qpool,
            tc.tile_pool(name="ap", bufs=2) as apool,
        ):
            def proj_load(src_ap, blk):
                sl = slice(512 * blk, 512 * blk + 512)
                act = work.tile([128, 4, 512], BF16, tag="act")
                nc.sync.dma_start(act[:], src_ap.rearrange("(o p) m -> p o m", p=128)[:, :, sl])
                return act

            def proj_rope(act, w_sb, b_sb, dst, dsl, blk, with_v):
                sl = slice(512 * blk, 512 * blk + 512)
                ps = sp.tile([128, 512], F32, tag="ps")
                for c in range(4):
                    nc.tensor.matmul(ps[:], w_sb[:, c, :], act[:, c, :],
                                     start=(c == 0), stop=(c == 3))
                kb = work.tile([128, 512], BF16, tag="kb")
                nc.vector.tensor_add(out=kb[:], in0=ps[:], in1=b_sb.to_broadcast((128, 512)))
                pr = sp.tile([128, 512], F32, tag="ps")
                nc.tensor.matmul(pr[:], r2_sb[:], kb[:], start=True, stop=True)
                t1 = work.tile([128, 512], F32, tag="t1")
                nc.vector.tensor_mul(out=t1[:], in0=kb[:], in1=cos_sb[:, sl])
                t2 = work.tile([128, 512], F32, tag="t2")
                nc.vector.tensor_mul(out=t2[:], in0=pr[:], in1=sin_sb[:, sl])
                nc.vector.tensor_add(out=dst[:, dsl], in0=t1[:], in1=t2[:])
                if with_v:
                    for mm in range(4):
                        pv = op.tile([128, 128], F32, tag="po")
                        for c in range(4):
                            nc.tensor.matmul(pv[:], act[:, c, 128 * mm:128 * mm + 128],
                                             wv_sb[:, c, :], start=(c == 0), stop=(c == 3))
                        mci = 4 * blk + mm
                        nc.vector.tensor_add(out=Vp0[:, mci, 0:64], in0=pv[:, 0:64],
                                             in1=bv_sb[:, 0:64])
                        nc.vector.tensor_add(out=Vp1[:, mci, 0:64], in0=pv[:, 64:128],
                                             in1=bv_sb[:, 64:128])

            kv_acts = {mb: proj_load(ctxT, mb) for mb in range(2)}
            for mb in range(MB):
                if mb + 2 < MB:
                    kv_acts[mb + 2] = proj_load(ctxT, mb + 2)
                proj_rope(kv_acts.pop(mb), wk_sb, bk_sb, KT,
                          slice(512 * mb, 512 * mb + 512), mb, with_v=True)
            qts = {}
            qts[0] = qpool.tile([128, 512], BF16, tag="qt", name="qt0")
            proj_rope(proj_load(xT, 0), wq_sb, bq_sb, qts[0], slice(0, 512), 0,
                      with_v=False)
            q_acts = {}
            oproj_pending = None

            def emit_oproj(pnb, patt):
                for nn in range(4):
                    rsl = slice(512 * pnb + 128 * nn, 512 * pnb + 128 * nn + 128)
                    po = op.tile([128, 512], F32, tag="po")
                    nc.tensor.matmul(po[:], patt[:, 128 * nn:128 * nn + 128], wo_sb[:],
                                     start=True, stop=True)
                    ob = nw.tile([128, 512], F32, tag="ob")
                    nc.vector.tensor_copy(out=ob[:], in_=po[:])
                    nc.sync.dma_start(out[rsl, :], ob[:])

            # ---- attention + output projection, software-pipelined ----
            for nb in range(NB):
                qt = qts.pop(nb)
                att = apool.tile([128, 512], BF16, tag="att")
                pv0 = vp.tile([128, 512], F32, tag="pv")
                pv1 = vp.tile([128, 512], F32, tag="pv")
                prev = None
                for mc in range(MC + 1):
                    ps01 = None
                    if mc < MC:
                        mcs = slice(128 * mc, 128 * mc + 128)
                        ps01 = sp.tile([128, 1024], F32, tag="ps")
                        nc.tensor.matmul(ps01[:, 0:512], KT[0:64, mcs], qt[0:64, :],
                                         start=True, stop=True, tile_position=(0, 0))
                        nc.tensor.matmul(ps01[:, 512:1024], KT[64:128, mcs], qt[64:128, :],
                                         start=True, stop=True, tile_position=(64, 0))
                    if prev is not None:
                        pmc, pps = prev
                        e01 = ew.tile([128, 1024], BF16, tag="e")
                        nc.scalar.activation(e01[:], pps[:], EXP, scale=SCALE)
                        nc.tensor.matmul(pv0[0:65, :], Vp0[:, pmc, :], e01[:, 0:512],
                                         start=(pmc == 0), stop=(pmc == MC - 1))
                        nc.tensor.matmul(pv1[0:65, :], Vp1[:, pmc, :], e01[:, 512:1024],
                                         start=(pmc == 0), stop=(pmc == MC - 1))
                    # next query block's projection, tucked mid-loop to fill PE idle
                    if mc == 0 and nb + 1 < NB:
                        q_acts[nb + 1] = proj_load(xT, nb + 1)
                    if mc == 6 and nb + 1 < NB:
                        qts[nb + 1] = qpool.tile([128, 512], BF16, tag="qt", name=f"qt{nb+1}")
                        proj_rope(q_acts.pop(nb + 1), wq_sb, bq_sb, qts[nb + 1],
                                  slice(0, 512), nb + 1, with_v=False)
                    if mc == 12 and oproj_pending is not None:
                        emit_oproj(*oproj_pending)
                        oproj_pending = None
                    prev = (mc, ps01) if mc < MC else None
                # normalize:  att = num * (1/denom), denom broadcast via DRAM roundtrip
                rec0 = nw.tile([128, 512], F32, tag="rec0")
                nc.vector.reciprocal(rec0[64:65, :], pv0[64:65, :])
                rec1 = nw.tile([128, 512], F32, tag="rec1")
                nc.vector.reciprocal(rec1[64:65, :], pv1[64:65, :])
                nc.sync.dma_start(scr[2 * nb:2 * nb + 1, :], rec0[64:65, :])
                nc.sync.dma_start(scr[2 * nb + 1:2 * nb + 2, :], rec1[64:65, :])
                bc = nw.tile([128, 512], F32, tag="bc")
                nc.sync.dma_start(bc[0:64, :], scr[2 * nb:2 * nb + 1, :].to_broadcast((64, 512)))
                nc.sync.dma_start(bc[64:128, :], scr[2 * nb + 1:2 * nb + 2, :].to_broadcast((64, 512)))
                nc.vector.tensor_mul(out=att[0:64, :], in0=pv0[0:64, :], in1=bc[0:64, :])
                nc.vector.tensor_mul(out=att[64:128, :], in0=pv1[0:64, :], in1=bc[64:128, :])
                # output projection deferred into the next block's loop
                if nb + 1 < NB:
                    oproj_pending = (nb, att)
                else:
                    emit_oproj(nb, att)


def build_program():
    nc = bacc.Bacc("TRN2", target_bir_lowering=False, debug=False)

    def din(name, shape, dt):
        return nc.dram_tensor(name, shape, dt, kind="ExternalInput").ap()

    aps = (
        din("xT", [C, N], BF16),
        din("ctxT", [C, M], BF16),
        din("wqT", [C, PJ], BF16),
        din("wkT", [C, PJ], BF16),
        din("wvT", [C, PJ], BF16),
        din("woT", [PJ, C], BF16),
        din("bqT", [PJ, 1], F32),
        din("bkT", [PJ, 1], F32),
        din("bv", [1, PJ], F32),
        din("cosT", [PJ, N], F32),
        din("sinT", [PJ, N], F32),
        din("r2T", [PJ, PJ], BF16),
        nc.dram_tensor("scr", [2 * NB, 512], F32).ap(),
        nc.dram_tensor("out", [N, C], F32, kind="ExternalOutput").ap(),
    )
    with tile.TileContext(nc) as tc:
        _build(tc, aps)
    nc.compile()
    return nc


_PROG = None


def _program():
    global _PROG
    if _PROG is None:
        _PROG = build_program()
    return _PROG


def rope_tables():
    idx = np.arange(0, D, 2, dtype=np.float32)
    inv_freq = 1.0 / (ROPE_BASE ** (idx / D))
    t = np.arange(N, dtype=np.float32)
    freqs = t[:, None] * inv_freq[None, :]          # (N, 32)
    emb = np.concatenate([freqs, freqs], axis=1)    # (N, 64)
    cos64 = np.cos(emb).T.astype(np.float32)        # (64, N)
    sin64 = np.sin(emb).T.astype(np.float32)
    cosT = np.ascontiguousarray(np.vstack([cos64, cos64]))
    sinT = np.ascontiguousarray(np.vstack([sin64, sin64]))
    return cosT, sinT


def r2t_matrix():
    R = np.zeros((D, D), np.float32)
    for i in range(D // 2):
        R[2 * i, 2 * i + 1] = -1.0
        R[2 * i + 1, 2 * i] = 1.0
    R2 = np.zeros((PJ, PJ), np.float32)
    R2[0:D, 0:D] = R
    R2[D:PJ, D:PJ] = R
    return np.ascontiguousarray(R2.T).astype(ml_dtypes.bfloat16)


def make_in_maps(x, context, Wq, bq, Wk, bk, Wv, bv, Wo):
    def bf(a):
        return np.ascontiguousarray(a).astype(ml_dtypes.bfloat16)

    def f32c(a):
        return np.ascontiguousarray(a, dtype=np.float32)

    cosT, sinT = rope_tables()
    r2T = r2t_matrix()
    xTb = [bf(x[b].T) for b in range(B)]
    ctxTb = [bf(context[b].T) for b in range(B)]
    in_maps = []
    for core in range(NCORES):
        b, p = core // 4, core % 4
        sl = slice(PJ * p, PJ * p + PJ)
        in_maps.append({
            "xT": xTb[b],
            "ctxT": ctxTb[b],
            "wqT": bf(Wq[sl, :].T),
            "wkT": bf(Wk[sl, :].T),
            "wvT": bf(Wv[sl, :].T),
            "woT": bf(Wo[:, sl].T),
            "bqT": f32c(bq[sl].reshape(PJ, 1)),
            "bkT": f32c(bk[sl].reshape(PJ, 1)),
            "bv": f32c(bv[sl].reshape(1, PJ)),
            "cosT": cosT,
            "sinT": sinT,
            "r2T": r2T,
        })
    return in_maps


def gather(partials, bo):
    final = np.empty((B, N, C), np.float32)
    for b in range(B):
        acc = partials[4 * b].astype(np.float32).copy()
        for p in range(1, 4):
            acc += partials[4 * b + p]
        final[b] = acc + np.asarray(bo, np.float32)[None, :]
    return final


def kernel(x, context, Wq, bq, Wk, bk, Wv, bv, Wo, bo, **kw):
    x = np.asarray(x, np.float32)
    context = np.asarray(context, np.float32)
    nc = _program()
    in_maps = make_in_maps(x, context, np.asarray(Wq, np.float32), np.asarray(bq, np.float32),
                           np.asarray(Wk, np.float32), np.asarray(bk, np.float32),
                           np.asarray(Wv, np.float32), np.asarray(bv, np.float32),
                           np.asarray(Wo, np.float32))
    res = run_bass_kernel_spmd(nc, in_maps, list(range(NCORES)))
    partials = [res.results[i]["out"] for i in range(NCORES)]
    return gather(partials, np.asarray(bo, np.float32))
